# revision 9
# baseline (speedup 1.0000x reference)
"""Trainium2 Bass kernel for nn_AggregationFusion (gnn_message_passing).

Computation (per node row i):
    sel    = aggr_nodes[searchsorted(aggr_comps, comps[i])]        # gather
    x      = concat([nodes[i], sel])                               # [2F]
    h      = LN1(x);  h = silu(h @ W1 + b1)
    h      = LN2(h);  out = silu(h @ W2 + b2)

Strategy: data-parallel over nodes across 8 NeuronCores (weights + gather
table replicated). Rows padded 100000 -> 100352 = 8 * 12544 = 8 * 98 * 128.
Per core, 98 row-tiles of 128 rows:
  - indirect-DMA gather of aggr rows, LN row-major (DVE bn_stats),
  - PE transpose to feature-major, matmuls in MM_DT (float32r is full-rate
    at free dim >= 256; plain fp32 runs at 1/4 rate),
  - biases folded in via a K=1 ones-row matmul; LN affine folded into
    weights on the host.
"""

import numpy as np

N_FULL = 100000
F = 512
TWO_F = 1024
M_TABLE = 16384
N_CORES = 8
ROWS_PER_CORE = 12544  # 98 tiles of 128
N_PAD = N_CORES * ROWS_PER_CORE
LN_EPS = 1e-5
MM_DT = "float32r"  # matmul operand dtype: "float32r" or "bfloat16"

_CACHE = {}


def _build(rows, act="Silu", mm_dt=MM_DT, wbufs=3, ptb=3, p1b=3, p2b=2):
    import concourse.bass as bass
    import concourse.tile as tile
    from concourse import bacc, mybir
    from concourse.masks import make_identity

    f32 = mybir.dt.float32
    i32 = mybir.dt.int32
    mdt = getattr(mybir.dt, mm_dt)
    AF = mybir.ActivationFunctionType
    OP = mybir.AluOpType
    ACT = getattr(AF, act)

    n_tiles = rows // 128
    assert rows % 128 == 0

    nc = bacc.Bacc("TRN2", target_bir_lowering=False, debug=False,
                   num_devices=N_CORES)
    nodes = nc.dram_tensor("nodes", [rows, F], f32, kind="ExternalInput").ap()
    idx = nc.dram_tensor("idx", [128, n_tiles], i32, kind="ExternalInput").ap()
    aggr = nc.dram_tensor("aggr", [M_TABLE, F], f32, kind="ExternalInput").ap()
    w1 = nc.dram_tensor("w1", [TWO_F, TWO_F], mdt, kind="ExternalInput").ap()
    c1 = nc.dram_tensor("c1", [1, TWO_F], mdt, kind="ExternalInput").ap()
    w2 = nc.dram_tensor("w2", [TWO_F, F], mdt, kind="ExternalInput").ap()
    c2 = nc.dram_tensor("c2", [1, F], mdt, kind="ExternalInput").ap()
    ones_d = nc.dram_tensor("ones", [1, 128], mdt, kind="ExternalInput").ap()
    out = nc.dram_tensor("out", [rows, F], f32, kind="ExternalOutput").ap()

    with tile.TileContext(nc) as tc:
        with (
            tc.tile_pool(name="const", bufs=1) as cpool,
            tc.tile_pool(name="work", bufs=wbufs) as wpool,
            tc.tile_pool(name="psT", bufs=ptb, space="PSUM") as ptpool,
            tc.tile_pool(name="ps1", bufs=p1b, space="PSUM") as p1pool,
            tc.tile_pool(name="ps2", bufs=p2b, space="PSUM") as p2pool,
        ):
            ident = cpool.tile([128, 128], f32, tag="ident")
            make_identity(nc, ident[:])
            ones1 = cpool.tile([1, 128], mdt, tag="ones")
            nc.sync.dma_start(ones1[:], ones_d[:, :])

            w1sb = []
            for k in range(8):
                t = cpool.tile([128, TWO_F], mdt, tag=f"w1_{k}")
                nc.sync.dma_start(t[:], w1[k * 128:(k + 1) * 128, :])
                w1sb.append(t)
            w2sb = []
            for k in range(8):
                t = cpool.tile([128, F], mdt, tag=f"w2_{k}")
                nc.sync.dma_start(t[:], w2[k * 128:(k + 1) * 128, :])
                w2sb.append(t)
            c1sb = cpool.tile([1, TWO_F], mdt, tag="c1")
            nc.sync.dma_start(c1sb[:], c1[:, :])
            c2sb = cpool.tile([1, F], mdt, tag="c2")
            nc.sync.dma_start(c2sb[:], c2[:, :])
            idxsb = cpool.tile([128, n_tiles], i32, tag="idx")
            nc.sync.dma_start(idxsb[:], idx[:, :])

            def layer_norm(src, dst, tagp):
                """LN stats+apply row-major: dst = (src - mean) * rsqrt(var+eps).

                rsqrt runs entirely on DVE (bit-trick seed + 2 Newton steps)
                so the scalar engine's activation table never leaves the
                silu set (a table swap costs ~1.3us)."""
                st = wpool.tile([128, 12], f32, tag=f"st_{tagp}")
                nc.vector.bn_stats(st[:, 0:6], src[:, 0:F])
                nc.vector.bn_stats(st[:, 6:12], src[:, F:TWO_F])
                mv = wpool.tile([128, 2], f32, tag=f"mv_{tagp}")
                nc.vector.bn_aggr(mv[:], st[:])
                ve = wpool.tile([128, 1], f32, tag=f"ve_{tagp}")
                nc.vector.tensor_scalar_add(ve[:], mv[:, 1:2], LN_EPS)
                yi = wpool.tile([128, 1], i32, tag=f"yi_{tagp}")
                nc.vector.tensor_scalar(yi[:], ve[:].bitcast(i32), 1, None,
                                        OP.arith_shift_right)
                # magic - t  ==  (~t) + (magic + 1)
                nc.vector.tensor_scalar(yi[:], yi[:], -1, 0x5F375A87,
                                        OP.bitwise_xor, OP.add)
                y = yi[:].bitcast(f32)
                for itn in range(2):
                    t = wpool.tile([128, 1], f32, tag=f"nr{itn}_{tagp}")
                    nc.vector.tensor_tensor(t[:], y, y, op=OP.mult)
                    nc.vector.scalar_tensor_tensor(t[:], t[:], -0.5, ve[:],
                                                   op0=OP.mult, op1=OP.mult)
                    nc.vector.tensor_scalar_add(t[:], t[:], 1.5)
                    yn = wpool.tile([128, 1], f32, tag=f"ny{itn}_{tagp}")
                    nc.vector.tensor_tensor(yn[:], y, t[:], op=OP.mult)
                    y = yn[:]
                nc.vector.tensor_scalar(dst[:], src[:], mv[:, 0:1], y,
                                        OP.subtract, OP.mult)

            def transpose_1024(src, dst, tagp):
                """dst[:, k*128:(k+1)*128] = src[:, k*128:(k+1)*128].T for k in 0..8.
                dst rounds to the matmul dtype."""
                for h in range(2):
                    pt = ptpool.tile([128, 512], f32, tag="pt")
                    for j in range(4):
                        c = 4 * h + j
                        nc.tensor.transpose(pt[:, j * 128:(j + 1) * 128],
                                            src[:, c * 128:(c + 1) * 128],
                                            ident[:])
                    nc.scalar.copy(dst[:, h * 512:(h + 1) * 512], pt[:])

            def stage_a(it):
                """load + gather -> LN1 -> transpose -> mm1 -> silu -> h1."""
                r0 = it * 128
                xt = wpool.tile([128, TWO_F], f32, tag="x")
                nc.sync.dma_start(xt[:, 0:F], nodes[r0:r0 + 128, :])
                nc.gpsimd.indirect_dma_start(
                    out=xt[:, F:TWO_F],
                    out_offset=None,
                    in_=aggr[:, :],
                    in_offset=bass.IndirectOffsetOnAxis(
                        ap=idxsb[:, it:it + 1], axis=0),
                )
                xh = wpool.tile([128, TWO_F], f32, tag="xh")
                layer_norm(xt, xh, "ln1")
                xT = wpool.tile([128, TWO_F], mdt, tag="xT")
                transpose_1024(xh, xT, "x")
                h1 = wpool.tile([128, TWO_F], f32, tag="h1")
                for n in range(2):
                    ps = p1pool.tile([128, 512], f32, tag="ps1")
                    for k in range(8):
                        nc.tensor.matmul(
                            ps[:],
                            xT[:, k * 128:(k + 1) * 128],
                            w1sb[k][:, n * 512:(n + 1) * 512],
                            start=(k == 0), stop=False)
                    nc.tensor.matmul(
                        ps[:], ones1[:1, :],
                        c1sb[:1, n * 512:(n + 1) * 512],
                        start=False, stop=True)
                    nc.scalar.activation(h1[:, n * 512:(n + 1) * 512], ps[:],
                                         ACT)
                return h1

            def stage_b(it, h1):
                """LN2 -> transpose -> mm2 -> silu -> store."""
                r0 = it * 128
                hh = wpool.tile([128, TWO_F], f32, tag="hh")
                layer_norm(h1, hh, "ln2")
                hT = wpool.tile([128, TWO_F], mdt, tag="hT")
                transpose_1024(hh, hT, "h")
                ps2 = p2pool.tile([128, F], f32, tag="ps2")
                for k in range(8):
                    nc.tensor.matmul(
                        ps2[:],
                        hT[:, k * 128:(k + 1) * 128],
                        w2sb[k][:],
                        start=(k == 0), stop=False)
                nc.tensor.matmul(ps2[:], ones1[:1, :],
                                 c2sb[:1, :],
                                 start=False, stop=True)
                ot = wpool.tile([128, F], f32, tag="ot")
                nc.scalar.activation(ot[:], ps2[:], ACT)
                nc.sync.dma_start(out[r0:r0 + 128, :], ot[:])

            # Software-pipelined: stage_b of tile i is emitted after stage_a
            # of tile i+1, so the PE fills the LN2 dependency gap of tile i
            # with tile i+1's transposes + mm1.
            h1_prev = None
            for it in range(n_tiles):
                h1_cur = stage_a(it)
                if h1_prev is not None:
                    stage_b(it - 1, h1_prev)
                h1_prev = h1_cur
            stage_b(n_tiles - 1, h1_prev)

    nc.compile()
    return nc


def _get_nc(rows):
    if rows not in _CACHE:
        _CACHE[rows] = _build(rows)
    return _CACHE[rows]


def _mm_np_dtype():
    if MM_DT == "bfloat16":
        import ml_dtypes
        return ml_dtypes.bfloat16
    return np.float32


def _host_prep(comps, aggr_comps, ln1_g, ln1_b, W1, b1, ln2_g, ln2_b, W2, b2):
    """Fold LN affine params into the weights; resolve gather indices."""
    idx = np.searchsorted(np.asarray(aggr_comps), np.asarray(comps)).astype(np.int32)
    W1 = np.asarray(W1, np.float32)
    W2 = np.asarray(W2, np.float32)
    dt = _mm_np_dtype()
    w1p = np.ascontiguousarray(
        (np.asarray(ln1_g, np.float32)[:, None] * W1).astype(dt))
    c1 = np.ascontiguousarray(
        (np.asarray(b1, np.float32) + np.asarray(ln1_b, np.float32) @ W1
         )[None].astype(dt))
    w2p = np.ascontiguousarray(
        (np.asarray(ln2_g, np.float32)[:, None] * W2).astype(dt))
    c2 = np.ascontiguousarray(
        (np.asarray(b2, np.float32) + np.asarray(ln2_b, np.float32) @ W2
         )[None].astype(dt))
    return idx, w1p, c1, w2p, c2


def _make_in_maps(nodes, comps, aggr_nodes, aggr_comps,
                  ln1_g, ln1_b, W1, b1, ln2_g, ln2_b, W2, b2):
    nodes = np.asarray(nodes, np.float32)
    aggr_nodes = np.ascontiguousarray(np.asarray(aggr_nodes, np.float32))
    idx, w1p, c1, w2p, c2 = _host_prep(
        comps, aggr_comps, ln1_g, ln1_b, W1, b1, ln2_g, ln2_b, W2, b2)
    n = nodes.shape[0]
    if n < N_PAD:
        nodes_p = np.zeros((N_PAD, F), np.float32)
        nodes_p[:n] = nodes
        idx_p = np.zeros((N_PAD,), np.int32)
        idx_p[:n] = idx
    else:
        nodes_p, idx_p = nodes, idx
    n_tiles = ROWS_PER_CORE // 128
    dt = _mm_np_dtype()
    in_maps = []
    for c in range(N_CORES):
        sl = slice(c * ROWS_PER_CORE, (c + 1) * ROWS_PER_CORE)
        in_maps.append({
            "nodes": np.ascontiguousarray(nodes_p[sl]),
            "idx": np.ascontiguousarray(idx_p[sl].reshape(n_tiles, 128).T),
            "aggr": aggr_nodes,
            "w1": w1p, "c1": c1, "w2": w2p, "c2": c2,
            "ones": np.ones((1, 128), dt),
        })
    return in_maps, n


def kernel(coords, nodes, comps, aggr_coords, aggr_nodes, aggr_comps,
           ln1_g, ln1_b, W1, b1, ln2_g, ln2_b, W2, b2):
    from concourse.bass_utils import run_bass_kernel_spmd

    in_maps, n = _make_in_maps(nodes, comps, aggr_nodes, aggr_comps,
                               ln1_g, ln1_b, W1, b1, ln2_g, ln2_b, W2, b2)
    nc = _get_nc(ROWS_PER_CORE)
    res = run_bass_kernel_spmd(nc, in_maps, list(range(N_CORES)))
    out = np.concatenate([res.results[c]["out"] for c in range(N_CORES)], axis=0)
    return out[:n]


# revision 10
# speedup vs baseline: 31.7372x; 31.7372x over previous
"""Trainium2 Bass kernel for nn_AggregationFusion (gnn_message_passing).

Computation (per node row i):
    sel    = aggr_nodes[searchsorted(aggr_comps, comps[i])]        # gather
    x      = concat([nodes[i], sel])                               # [2F]
    h      = LN1(x);  h = silu(h @ W1 + b1)
    h      = LN2(h);  out = silu(h @ W2 + b2)

Strategy: data-parallel over nodes across 8 NeuronCores (weights + gather
table replicated). Rows padded 100000 -> 100352 = 8 * 12544 = 8 * 98 * 128.
Per core, 98 row-tiles of 128 rows:
  - indirect-DMA gather of aggr rows, LN row-major (DVE bn_stats),
  - PE transpose to feature-major, matmuls in MM_DT (float32r is full-rate
    at free dim >= 256; plain fp32 runs at 1/4 rate),
  - biases folded in via a K=1 ones-row matmul; LN affine folded into
    weights on the host.
"""

import numpy as np

N_FULL = 100000
F = 512
TWO_F = 1024
M_TABLE = 16384
N_CORES = 8
ROWS_PER_CORE = 12544  # 98 tiles of 128
N_PAD = N_CORES * ROWS_PER_CORE
LN_EPS = 1e-5
MM_DT = "float32r"  # matmul operand dtype: "float32r" or "bfloat16"

_CACHE = {}


def _build(rows, act="Silu", mm_dt=MM_DT, wbufs=3, ptb=3, p1b=3, p2b=2):
    import concourse.bass as bass
    import concourse.tile as tile
    from concourse import bacc, mybir
    from concourse.masks import make_identity

    f32 = mybir.dt.float32
    i32 = mybir.dt.int32
    mdt = getattr(mybir.dt, mm_dt)
    AF = mybir.ActivationFunctionType
    OP = mybir.AluOpType
    ACT = getattr(AF, act)

    n_tiles = rows // 128
    assert rows % 128 == 0

    nc = bacc.Bacc("TRN2", target_bir_lowering=False, debug=False,
                   num_devices=N_CORES)
    nodes = nc.dram_tensor("nodes", [rows, F], f32, kind="ExternalInput").ap()
    idx = nc.dram_tensor("idx", [128, n_tiles], i32, kind="ExternalInput").ap()
    aggr = nc.dram_tensor("aggr", [M_TABLE, F], f32, kind="ExternalInput").ap()
    w1 = nc.dram_tensor("w1", [TWO_F, TWO_F], mdt, kind="ExternalInput").ap()
    c1 = nc.dram_tensor("c1", [1, TWO_F], mdt, kind="ExternalInput").ap()
    w2 = nc.dram_tensor("w2", [TWO_F, F], mdt, kind="ExternalInput").ap()
    c2 = nc.dram_tensor("c2", [1, F], mdt, kind="ExternalInput").ap()
    ones_d = nc.dram_tensor("ones", [1, 128], mdt, kind="ExternalInput").ap()
    out = nc.dram_tensor("out", [rows, F], f32, kind="ExternalOutput").ap()

    with tile.TileContext(nc) as tc:
        with (
            tc.tile_pool(name="const", bufs=1) as cpool,
            tc.tile_pool(name="work", bufs=wbufs) as wpool,
            tc.tile_pool(name="psT", bufs=ptb, space="PSUM") as ptpool,
            tc.tile_pool(name="ps1", bufs=p1b, space="PSUM") as p1pool,
            tc.tile_pool(name="ps2", bufs=p2b, space="PSUM") as p2pool,
        ):
            ident = cpool.tile([128, 128], f32, tag="ident")
            make_identity(nc, ident[:])
            ones1 = cpool.tile([1, 128], mdt, tag="ones")
            nc.sync.dma_start(ones1[:], ones_d[:, :])

            w1sb = []
            for k in range(8):
                t = cpool.tile([128, TWO_F], mdt, tag=f"w1_{k}")
                nc.sync.dma_start(t[:], w1[k * 128:(k + 1) * 128, :])
                w1sb.append(t)
            w2sb = []
            for k in range(8):
                t = cpool.tile([128, F], mdt, tag=f"w2_{k}")
                nc.sync.dma_start(t[:], w2[k * 128:(k + 1) * 128, :])
                w2sb.append(t)
            c1sb = cpool.tile([1, TWO_F], mdt, tag="c1")
            nc.sync.dma_start(c1sb[:], c1[:, :])
            c2sb = cpool.tile([1, F], mdt, tag="c2")
            nc.sync.dma_start(c2sb[:], c2[:, :])
            idxsb = cpool.tile([128, n_tiles], i32, tag="idx")
            nc.sync.dma_start(idxsb[:], idx[:, :])

            def layer_norm(src, dst, tagp):
                """LN stats+apply row-major: dst = (src - mean) * rsqrt(var+eps).

                rsqrt runs entirely on DVE (bit-trick seed + 2 Newton steps)
                so the scalar engine's activation table never leaves the
                silu set (a table swap costs ~1.3us)."""
                st = wpool.tile([128, 12], f32, tag=f"st_{tagp}")
                nc.vector.bn_stats(st[:, 0:6], src[:, 0:F])
                nc.vector.bn_stats(st[:, 6:12], src[:, F:TWO_F])
                mv = wpool.tile([128, 2], f32, tag=f"mv_{tagp}")
                nc.vector.bn_aggr(mv[:], st[:])
                ve = wpool.tile([128, 1], f32, tag=f"ve_{tagp}")
                nc.vector.tensor_scalar_add(ve[:], mv[:, 1:2], LN_EPS)
                yi = wpool.tile([128, 1], i32, tag=f"yi_{tagp}")
                nc.vector.tensor_scalar(yi[:], ve[:].bitcast(i32), 1, None,
                                        OP.arith_shift_right)
                # magic - t  ==  (~t) + (magic + 1)
                nc.vector.tensor_scalar(yi[:], yi[:], -1, None,
                                        OP.bitwise_xor)
                nc.vector.tensor_scalar(yi[:], yi[:], 0x5F375A87, None,
                                        OP.add)
                y = yi[:].bitcast(f32)
                for itn in range(2):
                    t = wpool.tile([128, 1], f32, tag=f"nr{itn}_{tagp}")
                    nc.vector.tensor_tensor(t[:], y, y, op=OP.mult)
                    nc.vector.scalar_tensor_tensor(t[:], t[:], -0.5, ve[:],
                                                   op0=OP.mult, op1=OP.mult)
                    nc.vector.tensor_scalar_add(t[:], t[:], 1.5)
                    yn = wpool.tile([128, 1], f32, tag=f"ny{itn}_{tagp}")
                    nc.vector.tensor_tensor(yn[:], y, t[:], op=OP.mult)
                    y = yn[:]
                nc.vector.tensor_scalar(dst[:], src[:], mv[:, 0:1], y,
                                        OP.subtract, OP.mult)

            def transpose_1024(src, dst, tagp):
                """dst[:, k*128:(k+1)*128] = src[:, k*128:(k+1)*128].T for k in 0..8.
                dst rounds to the matmul dtype."""
                for h in range(2):
                    pt = ptpool.tile([128, 512], f32, tag="pt")
                    for j in range(4):
                        c = 4 * h + j
                        nc.tensor.transpose(pt[:, j * 128:(j + 1) * 128],
                                            src[:, c * 128:(c + 1) * 128],
                                            ident[:])
                    nc.scalar.copy(dst[:, h * 512:(h + 1) * 512], pt[:])

            def stage_a(it):
                """load + gather -> LN1 -> transpose -> mm1 -> silu -> h1."""
                r0 = it * 128
                xt = wpool.tile([128, TWO_F], f32, tag="x")
                nc.sync.dma_start(xt[:, 0:F], nodes[r0:r0 + 128, :])
                nc.gpsimd.indirect_dma_start(
                    out=xt[:, F:TWO_F],
                    out_offset=None,
                    in_=aggr[:, :],
                    in_offset=bass.IndirectOffsetOnAxis(
                        ap=idxsb[:, it:it + 1], axis=0),
                )
                xh = wpool.tile([128, TWO_F], f32, tag="xh")
                layer_norm(xt, xh, "ln1")
                xT = wpool.tile([128, TWO_F], mdt, tag="xT")
                transpose_1024(xh, xT, "x")
                h1 = wpool.tile([128, TWO_F], f32, tag="h1")
                for n in range(2):
                    ps = p1pool.tile([128, 512], f32, tag="ps1")
                    for k in range(8):
                        nc.tensor.matmul(
                            ps[:],
                            xT[:, k * 128:(k + 1) * 128],
                            w1sb[k][:, n * 512:(n + 1) * 512],
                            start=(k == 0), stop=False)
                    nc.tensor.matmul(
                        ps[:], ones1[:1, :],
                        c1sb[:1, n * 512:(n + 1) * 512],
                        start=False, stop=True)
                    nc.scalar.activation(h1[:, n * 512:(n + 1) * 512], ps[:],
                                         ACT)
                return h1

            def stage_b(it, h1):
                """LN2 -> transpose -> mm2 -> silu -> store."""
                r0 = it * 128
                hh = wpool.tile([128, TWO_F], f32, tag="hh")
                layer_norm(h1, hh, "ln2")
                hT = wpool.tile([128, TWO_F], mdt, tag="hT")
                transpose_1024(hh, hT, "h")
                ps2 = p2pool.tile([128, F], f32, tag="ps2")
                for k in range(8):
                    nc.tensor.matmul(
                        ps2[:],
                        hT[:, k * 128:(k + 1) * 128],
                        w2sb[k][:],
                        start=(k == 0), stop=False)
                nc.tensor.matmul(ps2[:], ones1[:1, :],
                                 c2sb[:1, :],
                                 start=False, stop=True)
                ot = wpool.tile([128, F], f32, tag="ot")
                nc.scalar.activation(ot[:], ps2[:], ACT)
                nc.sync.dma_start(out[r0:r0 + 128, :], ot[:])

            # Software-pipelined: stage_b of tile i is emitted after stage_a
            # of tile i+1, so the PE fills the LN2 dependency gap of tile i
            # with tile i+1's transposes + mm1.
            h1_prev = None
            for it in range(n_tiles):
                h1_cur = stage_a(it)
                if h1_prev is not None:
                    stage_b(it - 1, h1_prev)
                h1_prev = h1_cur
            stage_b(n_tiles - 1, h1_prev)

    nc.compile()
    return nc


def _get_nc(rows):
    if rows not in _CACHE:
        _CACHE[rows] = _build(rows)
    return _CACHE[rows]


def _mm_np_dtype():
    if MM_DT == "bfloat16":
        import ml_dtypes
        return ml_dtypes.bfloat16
    return np.float32


def _host_prep(comps, aggr_comps, ln1_g, ln1_b, W1, b1, ln2_g, ln2_b, W2, b2):
    """Fold LN affine params into the weights; resolve gather indices."""
    idx = np.searchsorted(np.asarray(aggr_comps), np.asarray(comps)).astype(np.int32)
    W1 = np.asarray(W1, np.float32)
    W2 = np.asarray(W2, np.float32)
    dt = _mm_np_dtype()
    w1p = np.ascontiguousarray(
        (np.asarray(ln1_g, np.float32)[:, None] * W1).astype(dt))
    c1 = np.ascontiguousarray(
        (np.asarray(b1, np.float32) + np.asarray(ln1_b, np.float32) @ W1
         )[None].astype(dt))
    w2p = np.ascontiguousarray(
        (np.asarray(ln2_g, np.float32)[:, None] * W2).astype(dt))
    c2 = np.ascontiguousarray(
        (np.asarray(b2, np.float32) + np.asarray(ln2_b, np.float32) @ W2
         )[None].astype(dt))
    return idx, w1p, c1, w2p, c2


def _make_in_maps(nodes, comps, aggr_nodes, aggr_comps,
                  ln1_g, ln1_b, W1, b1, ln2_g, ln2_b, W2, b2):
    nodes = np.asarray(nodes, np.float32)
    aggr_nodes = np.ascontiguousarray(np.asarray(aggr_nodes, np.float32))
    idx, w1p, c1, w2p, c2 = _host_prep(
        comps, aggr_comps, ln1_g, ln1_b, W1, b1, ln2_g, ln2_b, W2, b2)
    n = nodes.shape[0]
    if n < N_PAD:
        nodes_p = np.zeros((N_PAD, F), np.float32)
        nodes_p[:n] = nodes
        idx_p = np.zeros((N_PAD,), np.int32)
        idx_p[:n] = idx
    else:
        nodes_p, idx_p = nodes, idx
    n_tiles = ROWS_PER_CORE // 128
    dt = _mm_np_dtype()
    in_maps = []
    for c in range(N_CORES):
        sl = slice(c * ROWS_PER_CORE, (c + 1) * ROWS_PER_CORE)
        in_maps.append({
            "nodes": np.ascontiguousarray(nodes_p[sl]),
            "idx": np.ascontiguousarray(idx_p[sl].reshape(n_tiles, 128).T),
            "aggr": aggr_nodes,
            "w1": w1p, "c1": c1, "w2": w2p, "c2": c2,
            "ones": np.ones((1, 128), dt),
        })
    return in_maps, n


def kernel(coords, nodes, comps, aggr_coords, aggr_nodes, aggr_comps,
           ln1_g, ln1_b, W1, b1, ln2_g, ln2_b, W2, b2):
    from concourse.bass_utils import run_bass_kernel_spmd

    in_maps, n = _make_in_maps(nodes, comps, aggr_nodes, aggr_comps,
                               ln1_g, ln1_b, W1, b1, ln2_g, ln2_b, W2, b2)
    nc = _get_nc(ROWS_PER_CORE)
    res = run_bass_kernel_spmd(nc, in_maps, list(range(N_CORES)))
    out = np.concatenate([res.results[c]["out"] for c in range(N_CORES)], axis=0)
    return out[:n]


# revision 11
# speedup vs baseline: 57.5124x; 1.8121x over previous
"""Trainium2 Bass kernel for nn_AggregationFusion (gnn_message_passing).

Computation (per node row i):
    sel    = aggr_nodes[searchsorted(aggr_comps, comps[i])]        # gather
    x      = concat([nodes[i], sel])                               # [2F]
    h      = LN1(x);  h = silu(h @ W1 + b1)
    h      = LN2(h);  out = silu(h @ W2 + b2)

Strategy: data-parallel over nodes across 8 NeuronCores (weights + gather
table replicated). Rows padded 100000 -> 100352 = 8 * 12544 = 8 * 98 * 128.
Per core, 98 row-tiles of 128 rows:
  - indirect-DMA gather of aggr rows, LN row-major (DVE bn_stats),
  - PE transpose to feature-major, matmuls in MM_DT (float32r is full-rate
    at free dim >= 256; plain fp32 runs at 1/4 rate),
  - biases folded in via a K=1 ones-row matmul; LN affine folded into
    weights on the host.
"""

import numpy as np

N_FULL = 100000
F = 512
TWO_F = 1024
M_TABLE = 16384
N_CORES = 8
ROWS_PER_CORE = 12544  # 98 tiles of 128
N_PAD = N_CORES * ROWS_PER_CORE
LN_EPS = 1e-5
MM_DT = "bfloat16"  # matmul operand dtype: "float32r" or "bfloat16"

_CACHE = {}


def _build(rows, act="Silu", mm_dt=MM_DT, wbufs=3, ptb=3, p1b=3, p2b=2):
    import concourse.bass as bass
    import concourse.tile as tile
    from concourse import bacc, mybir
    from concourse.masks import make_identity

    f32 = mybir.dt.float32
    i32 = mybir.dt.int32
    mdt = getattr(mybir.dt, mm_dt)
    AF = mybir.ActivationFunctionType
    OP = mybir.AluOpType
    ACT = getattr(AF, act)

    n_tiles = rows // 128
    assert rows % 128 == 0

    nc = bacc.Bacc("TRN2", target_bir_lowering=False, debug=False,
                   num_devices=N_CORES)
    nodes = nc.dram_tensor("nodes", [rows, F], f32, kind="ExternalInput").ap()
    idx = nc.dram_tensor("idx", [128, n_tiles], i32, kind="ExternalInput").ap()
    aggr = nc.dram_tensor("aggr", [M_TABLE, F], f32, kind="ExternalInput").ap()
    w1 = nc.dram_tensor("w1", [TWO_F, TWO_F], mdt, kind="ExternalInput").ap()
    c1 = nc.dram_tensor("c1", [1, TWO_F], mdt, kind="ExternalInput").ap()
    w2 = nc.dram_tensor("w2", [TWO_F, F], mdt, kind="ExternalInput").ap()
    c2 = nc.dram_tensor("c2", [1, F], mdt, kind="ExternalInput").ap()
    ones_d = nc.dram_tensor("ones", [1, 128], mdt, kind="ExternalInput").ap()
    out = nc.dram_tensor("out", [rows, F], f32, kind="ExternalOutput").ap()

    with tile.TileContext(nc) as tc:
        with (
            tc.tile_pool(name="const", bufs=1) as cpool,
            tc.tile_pool(name="work", bufs=wbufs) as wpool,
            tc.tile_pool(name="psT", bufs=ptb, space="PSUM") as ptpool,
            tc.tile_pool(name="ps1", bufs=p1b, space="PSUM") as p1pool,
            tc.tile_pool(name="ps2", bufs=p2b, space="PSUM") as p2pool,
        ):
            ident = cpool.tile([128, 128], f32, tag="ident")
            make_identity(nc, ident[:])
            ones1 = cpool.tile([1, 128], mdt, tag="ones")
            nc.sync.dma_start(ones1[:], ones_d[:, :])

            w1sb = []
            for k in range(8):
                t = cpool.tile([128, TWO_F], mdt, tag=f"w1_{k}")
                nc.sync.dma_start(t[:], w1[k * 128:(k + 1) * 128, :])
                w1sb.append(t)
            w2sb = []
            for k in range(8):
                t = cpool.tile([128, F], mdt, tag=f"w2_{k}")
                nc.sync.dma_start(t[:], w2[k * 128:(k + 1) * 128, :])
                w2sb.append(t)
            c1sb = cpool.tile([1, TWO_F], mdt, tag="c1")
            nc.sync.dma_start(c1sb[:], c1[:, :])
            c2sb = cpool.tile([1, F], mdt, tag="c2")
            nc.sync.dma_start(c2sb[:], c2[:, :])
            idxsb = cpool.tile([128, n_tiles], i32, tag="idx")
            nc.sync.dma_start(idxsb[:], idx[:, :])

            def layer_norm(src, dst, tagp):
                """LN stats+apply row-major: dst = (src - mean) * rsqrt(var+eps).

                rsqrt runs entirely on DVE (bit-trick seed + 2 Newton steps)
                so the scalar engine's activation table never leaves the
                silu set (a table swap costs ~1.3us)."""
                st = wpool.tile([128, 12], f32, tag=f"st_{tagp}")
                nc.vector.bn_stats(st[:, 0:6], src[:, 0:F])
                nc.vector.bn_stats(st[:, 6:12], src[:, F:TWO_F])
                mv = wpool.tile([128, 2], f32, tag=f"mv_{tagp}")
                nc.vector.bn_aggr(mv[:], st[:])
                ve = wpool.tile([128, 1], f32, tag=f"ve_{tagp}")
                nc.vector.tensor_scalar_add(ve[:], mv[:, 1:2], LN_EPS)
                yi = wpool.tile([128, 1], i32, tag=f"yi_{tagp}")
                nc.vector.tensor_scalar(yi[:], ve[:].bitcast(i32), 1, None,
                                        OP.arith_shift_right)
                # magic - t  ==  (~t) + (magic + 1)
                nc.vector.tensor_scalar(yi[:], yi[:], -1, None,
                                        OP.bitwise_xor)
                nc.vector.tensor_scalar(yi[:], yi[:], 0x5F375A87, None,
                                        OP.add)
                y = yi[:].bitcast(f32)
                for itn in range(2):
                    t = wpool.tile([128, 1], f32, tag=f"nr{itn}_{tagp}")
                    nc.vector.tensor_tensor(t[:], y, y, op=OP.mult)
                    nc.vector.scalar_tensor_tensor(t[:], t[:], -0.5, ve[:],
                                                   op0=OP.mult, op1=OP.mult)
                    nc.vector.tensor_scalar_add(t[:], t[:], 1.5)
                    yn = wpool.tile([128, 1], f32, tag=f"ny{itn}_{tagp}")
                    nc.vector.tensor_tensor(yn[:], y, t[:], op=OP.mult)
                    y = yn[:]
                nc.vector.tensor_scalar(dst[:], src[:], mv[:, 0:1], y,
                                        OP.subtract, OP.mult)

            def transpose_1024(src, dst, tagp):
                """dst[:, k*128:(k+1)*128] = src[:, k*128:(k+1)*128].T for k in 0..8.
                dst rounds to the matmul dtype."""
                for h in range(2):
                    pt = ptpool.tile([128, 512], f32, tag="pt")
                    for j in range(4):
                        c = 4 * h + j
                        nc.tensor.transpose(pt[:, j * 128:(j + 1) * 128],
                                            src[:, c * 128:(c + 1) * 128],
                                            ident[:])
                    nc.scalar.copy(dst[:, h * 512:(h + 1) * 512], pt[:])

            def stage_a(it):
                """load + gather -> LN1 -> transpose -> mm1 -> silu -> h1."""
                r0 = it * 128
                xt = wpool.tile([128, TWO_F], f32, tag="x")
                nc.sync.dma_start(xt[:, 0:F], nodes[r0:r0 + 128, :])
                nc.gpsimd.indirect_dma_start(
                    out=xt[:, F:TWO_F],
                    out_offset=None,
                    in_=aggr[:, :],
                    in_offset=bass.IndirectOffsetOnAxis(
                        ap=idxsb[:, it:it + 1], axis=0),
                )
                xh = wpool.tile([128, TWO_F], f32, tag="xh")
                layer_norm(xt, xh, "ln1")
                xT = wpool.tile([128, TWO_F], mdt, tag="xT")
                transpose_1024(xh, xT, "x")
                h1 = wpool.tile([128, TWO_F], f32, tag="h1")
                for n in range(2):
                    ps = p1pool.tile([128, 512], f32, tag="ps1")
                    for k in range(8):
                        nc.tensor.matmul(
                            ps[:],
                            xT[:, k * 128:(k + 1) * 128],
                            w1sb[k][:, n * 512:(n + 1) * 512],
                            start=(k == 0), stop=False)
                    nc.tensor.matmul(
                        ps[:], ones1[:1, :],
                        c1sb[:1, n * 512:(n + 1) * 512],
                        start=False, stop=True)
                    nc.scalar.activation(h1[:, n * 512:(n + 1) * 512], ps[:],
                                         ACT)
                return h1

            def stage_b(it, h1):
                """LN2 -> transpose -> mm2 -> silu -> store."""
                r0 = it * 128
                hh = wpool.tile([128, TWO_F], f32, tag="hh")
                layer_norm(h1, hh, "ln2")
                hT = wpool.tile([128, TWO_F], mdt, tag="hT")
                transpose_1024(hh, hT, "h")
                ps2 = p2pool.tile([128, F], f32, tag="ps2")
                for k in range(8):
                    nc.tensor.matmul(
                        ps2[:],
                        hT[:, k * 128:(k + 1) * 128],
                        w2sb[k][:],
                        start=(k == 0), stop=False)
                nc.tensor.matmul(ps2[:], ones1[:1, :],
                                 c2sb[:1, :],
                                 start=False, stop=True)
                ot = wpool.tile([128, F], f32, tag="ot")
                nc.scalar.activation(ot[:], ps2[:], ACT)
                nc.sync.dma_start(out[r0:r0 + 128, :], ot[:])

            # Software-pipelined: stage_b of tile i is emitted after stage_a
            # of tile i+1, so the PE fills the LN2 dependency gap of tile i
            # with tile i+1's transposes + mm1.
            h1_prev = None
            for it in range(n_tiles):
                h1_cur = stage_a(it)
                if h1_prev is not None:
                    stage_b(it - 1, h1_prev)
                h1_prev = h1_cur
            stage_b(n_tiles - 1, h1_prev)

    nc.compile()
    return nc


def _get_nc(rows):
    if rows not in _CACHE:
        _CACHE[rows] = _build(rows)
    return _CACHE[rows]


def _mm_np_dtype():
    if MM_DT == "bfloat16":
        import ml_dtypes
        return ml_dtypes.bfloat16
    return np.float32


def _host_prep(comps, aggr_comps, ln1_g, ln1_b, W1, b1, ln2_g, ln2_b, W2, b2):
    """Fold LN affine params into the weights; resolve gather indices."""
    idx = np.searchsorted(np.asarray(aggr_comps), np.asarray(comps)).astype(np.int32)
    W1 = np.asarray(W1, np.float32)
    W2 = np.asarray(W2, np.float32)
    dt = _mm_np_dtype()
    w1p = np.ascontiguousarray(
        (np.asarray(ln1_g, np.float32)[:, None] * W1).astype(dt))
    c1 = np.ascontiguousarray(
        (np.asarray(b1, np.float32) + np.asarray(ln1_b, np.float32) @ W1
         )[None].astype(dt))
    w2p = np.ascontiguousarray(
        (np.asarray(ln2_g, np.float32)[:, None] * W2).astype(dt))
    c2 = np.ascontiguousarray(
        (np.asarray(b2, np.float32) + np.asarray(ln2_b, np.float32) @ W2
         )[None].astype(dt))
    return idx, w1p, c1, w2p, c2


def _make_in_maps(nodes, comps, aggr_nodes, aggr_comps,
                  ln1_g, ln1_b, W1, b1, ln2_g, ln2_b, W2, b2):
    nodes = np.asarray(nodes, np.float32)
    aggr_nodes = np.ascontiguousarray(np.asarray(aggr_nodes, np.float32))
    idx, w1p, c1, w2p, c2 = _host_prep(
        comps, aggr_comps, ln1_g, ln1_b, W1, b1, ln2_g, ln2_b, W2, b2)
    n = nodes.shape[0]
    if n < N_PAD:
        nodes_p = np.zeros((N_PAD, F), np.float32)
        nodes_p[:n] = nodes
        idx_p = np.zeros((N_PAD,), np.int32)
        idx_p[:n] = idx
    else:
        nodes_p, idx_p = nodes, idx
    n_tiles = ROWS_PER_CORE // 128
    dt = _mm_np_dtype()
    in_maps = []
    for c in range(N_CORES):
        sl = slice(c * ROWS_PER_CORE, (c + 1) * ROWS_PER_CORE)
        in_maps.append({
            "nodes": np.ascontiguousarray(nodes_p[sl]),
            "idx": np.ascontiguousarray(idx_p[sl].reshape(n_tiles, 128).T),
            "aggr": aggr_nodes,
            "w1": w1p, "c1": c1, "w2": w2p, "c2": c2,
            "ones": np.ones((1, 128), dt),
        })
    return in_maps, n


def kernel(coords, nodes, comps, aggr_coords, aggr_nodes, aggr_comps,
           ln1_g, ln1_b, W1, b1, ln2_g, ln2_b, W2, b2):
    from concourse.bass_utils import run_bass_kernel_spmd

    in_maps, n = _make_in_maps(nodes, comps, aggr_nodes, aggr_comps,
                               ln1_g, ln1_b, W1, b1, ln2_g, ln2_b, W2, b2)
    nc = _get_nc(ROWS_PER_CORE)
    res = run_bass_kernel_spmd(nc, in_maps, list(range(N_CORES)))
    out = np.concatenate([res.results[c]["out"] for c in range(N_CORES)], axis=0)
    return out[:n]


# revision 14
# speedup vs baseline: 60.6491x; 1.0545x over previous
"""Trainium2 Bass kernel for nn_AggregationFusion (gnn_message_passing).

Computation (per node row i):
    sel    = aggr_nodes[searchsorted(aggr_comps, comps[i])]        # gather
    x      = concat([nodes[i], sel])                               # [2F]
    h      = LN1(x);  h = silu(h @ W1 + b1)
    h      = LN2(h);  out = silu(h @ W2 + b2)

Strategy: data-parallel over nodes across 8 NeuronCores (weights + gather
table replicated). Rows padded 100000 -> 100352 = 8 * 12544 = 8 * 98 * 128.
Per core, 98 row-tiles of 128 rows.

LayerNorm is applied on the matmul OUTPUT side so the PE never waits for
the LN statistics:
    LN(v) @ W + c = (v@W + (-mu) x s + std x c) * inv       (per row)
with s = colsum(W), x = outer product, inv = rsqrt(var+eps), std = 1/inv.
The two rank-1 terms are one K=2 matmul whose stationary operand is the
transposed [-mu; std] pair; the final inv scale rides the SiLU activation
for free. LN gains/biases are folded into W and c on the host.
rsqrt runs on the vector engine (bit-trick + 2 Newton steps) so the scalar
engine's activation table never leaves the silu set (a swap costs ~1.3us).
"""

import numpy as np

N_FULL = 100000
F = 512
TWO_F = 1024
M_TABLE = 16384
N_CORES = 8
ROWS_PER_CORE = 12544  # 98 tiles of 128
N_PAD = N_CORES * ROWS_PER_CORE
LN_EPS = 1e-5
MM_DT = "bfloat16"  # matmul operand dtype: "float32r" or "bfloat16"

_CACHE = {}


def _build(rows, act="Silu", mm_dt=MM_DT, wbufs=3, ptb=2, p1b=3, p2b=2):
    import concourse.bass as bass
    import concourse.tile as tile
    from concourse import bacc, mybir
    from concourse.masks import make_identity

    f32 = mybir.dt.float32
    i32 = mybir.dt.int32
    mdt = getattr(mybir.dt, mm_dt)
    AF = mybir.ActivationFunctionType
    OP = mybir.AluOpType
    ACT = getattr(AF, act)

    n_tiles = rows // 128
    assert rows % 128 == 0

    nc = bacc.Bacc("TRN2", target_bir_lowering=False, debug=False,
                   num_devices=N_CORES)
    nodes = nc.dram_tensor("nodes", [rows, F], f32, kind="ExternalInput").ap()
    idx = nc.dram_tensor("idx", [128, n_tiles], i32, kind="ExternalInput").ap()
    aggr = nc.dram_tensor("aggr", [M_TABLE, F], f32, kind="ExternalInput").ap()
    w1 = nc.dram_tensor("w1", [TWO_F, TWO_F], mdt, kind="ExternalInput").ap()
    sc1 = nc.dram_tensor("sc1", [2, TWO_F], mdt, kind="ExternalInput").ap()
    w2 = nc.dram_tensor("w2", [TWO_F, F], mdt, kind="ExternalInput").ap()
    sc2 = nc.dram_tensor("sc2", [2, F], mdt, kind="ExternalInput").ap()
    out = nc.dram_tensor("out", [rows, F], f32, kind="ExternalOutput").ap()

    with tile.TileContext(nc) as tc:
        with (
            tc.tile_pool(name="const", bufs=1) as cpool,
            tc.tile_pool(name="work", bufs=wbufs) as wpool,
            tc.tile_pool(name="psT", bufs=ptb, space="PSUM") as ptpool,
            tc.tile_pool(name="psP", bufs=1, space="PSUM") as pppool,
            tc.tile_pool(name="ps1", bufs=p1b, space="PSUM") as p1pool,
            tc.tile_pool(name="ps2", bufs=p2b, space="PSUM") as p2pool,
        ):
            ident = cpool.tile([128, 128], f32, tag="ident")
            make_identity(nc, ident[:])

            w1sb = []
            for k in range(8):
                t = cpool.tile([128, TWO_F], mdt, tag=f"w1_{k}")
                nc.sync.dma_start(t[:], w1[k * 128:(k + 1) * 128, :])
                w1sb.append(t)
            w2sb = []
            for k in range(8):
                t = cpool.tile([128, F], mdt, tag=f"w2_{k}")
                nc.sync.dma_start(t[:], w2[k * 128:(k + 1) * 128, :])
                w2sb.append(t)
            sc1sb = cpool.tile([2, TWO_F], mdt, tag="sc1")
            nc.sync.dma_start(sc1sb[:], sc1[:, :])
            sc2sb = cpool.tile([2, F], mdt, tag="sc2")
            nc.sync.dma_start(sc2sb[:], sc2[:, :])
            idxsb = cpool.tile([128, n_tiles], i32, tag="idx")
            nc.sync.dma_start(idxsb[:], idx[:, :])

            def ln_stats(src, tagp):
                """Row-wise mean/var of src [128, 1024] -> (inv, pairT) where
                inv = rsqrt(var+eps) [128,1] f32 and pairT [2,128] mdt holds
                the transposed [-mean; 1/inv] pair (stationary operand of the
                rank-1 LN-correction matmul). All scalar math on DVE."""
                st = wpool.tile([128, 12], f32, tag=f"st_{tagp}")
                nc.vector.bn_stats(st[:, 0:6], src[:, 0:F])
                nc.vector.bn_stats(st[:, 6:12], src[:, F:TWO_F])
                mv = wpool.tile([128, 2], f32, tag=f"mv_{tagp}")
                nc.vector.bn_aggr(mv[:], st[:])
                ve = wpool.tile([128, 1], f32, tag=f"ve_{tagp}")
                nc.vector.tensor_scalar_add(ve[:], mv[:, 1:2], LN_EPS)
                yi = wpool.tile([128, 1], i32, tag=f"yi_{tagp}")
                nc.vector.tensor_scalar(yi[:], ve[:].bitcast(i32), 1, None,
                                        OP.arith_shift_right)
                # magic - t  ==  (~t) + (magic + 1)
                nc.vector.tensor_scalar(yi[:], yi[:], -1, None, OP.bitwise_xor)
                nc.vector.tensor_scalar(yi[:], yi[:], 0x5F375A87, None, OP.add)
                y = yi[:].bitcast(f32)
                for itn in range(2):
                    t = wpool.tile([128, 1], f32, tag=f"nr{itn}_{tagp}")
                    nc.vector.tensor_tensor(t[:], y, y, op=OP.mult)
                    nc.vector.scalar_tensor_tensor(t[:], t[:], -0.5, ve[:],
                                                   op0=OP.mult, op1=OP.mult)
                    nc.vector.tensor_scalar_add(t[:], t[:], 1.5)
                    yn = wpool.tile([128, 1], f32, tag=f"ny{itn}_{tagp}")
                    nc.vector.tensor_tensor(yn[:], y, t[:], op=OP.mult)
                    y = yn[:]
                pair = wpool.tile([128, 2], f32, tag=f"pair_{tagp}")
                nc.vector.tensor_scalar_mul(pair[:, 0:1], mv[:, 0:1], -1.0)
                nc.vector.tensor_tensor(pair[:, 1:2], ve[:], y, op=OP.mult)
                pp = pppool.tile([2, 128], f32, tag="pairT_ps")
                nc.tensor.transpose(pp[:], pair[:], ident[:])
                pairT = wpool.tile([2, 128], mdt, tag=f"pairT_{tagp}")
                nc.scalar.copy(pairT[:], pp[:])
                return y, pairT

            def transpose_1024(src, dst):
                """dst[:, k*128:(k+1)*128] = src[:, k*128:(k+1)*128].T,
                k = 0..8; dst rounds to the matmul dtype."""
                for h in range(2):
                    pt = ptpool.tile([128, 512], f32, tag="pt")
                    for j in range(4):
                        c = 4 * h + j
                        nc.tensor.transpose(pt[:, j * 128:(j + 1) * 128],
                                            src[:, c * 128:(c + 1) * 128],
                                            ident[:])
                    nc.scalar.copy(dst[:, h * 512:(h + 1) * 512], pt[:])

            def stage_a(it):
                """load + gather -> transpose raw x -> mm1 (+LN1 rank-1
                corrections) -> silu(scale=inv1) -> h1."""
                r0 = it * 128
                xt = wpool.tile([128, TWO_F], f32, tag="x")
                nc.sync.dma_start(xt[:, 0:F], nodes[r0:r0 + 128, :])
                nc.gpsimd.indirect_dma_start(
                    out=xt[:, F:TWO_F],
                    out_offset=None,
                    in_=aggr[:, :],
                    in_offset=bass.IndirectOffsetOnAxis(
                        ap=idxsb[:, it:it + 1], axis=0),
                )
                inv1, pairT1 = ln_stats(xt, "ln1")
                xT = wpool.tile([128, TWO_F], mdt, tag="xT")
                transpose_1024(xt, xT)
                h1 = wpool.tile([128, TWO_F], f32, tag="h1")
                for n in range(2):
                    ps = p1pool.tile([128, 512], f32, tag="ps1")
                    for k in range(8):
                        nc.tensor.matmul(
                            ps[:],
                            xT[:, k * 128:(k + 1) * 128],
                            w1sb[k][:, n * 512:(n + 1) * 512],
                            start=(k == 0), stop=False)
                    nc.tensor.matmul(
                        ps[:], pairT1[:2, :],
                        sc1sb[:2, n * 512:(n + 1) * 512],
                        start=False, stop=True)
                    nc.scalar.activation(h1[:, n * 512:(n + 1) * 512], ps[:],
                                         ACT, scale=inv1[:])
                return h1

            def stage_b(it, h1):
                """transpose raw h1 -> mm2 (+LN2 corrections) ->
                silu(scale=inv2) -> store."""
                r0 = it * 128
                inv2, pairT2 = ln_stats(h1, "ln2")
                hT = wpool.tile([128, TWO_F], mdt, tag="hT")
                transpose_1024(h1, hT)
                ps2 = p2pool.tile([128, F], f32, tag="ps2")
                for k in range(8):
                    nc.tensor.matmul(
                        ps2[:],
                        hT[:, k * 128:(k + 1) * 128],
                        w2sb[k][:],
                        start=(k == 0), stop=False)
                nc.tensor.matmul(ps2[:], pairT2[:2, :], sc2sb[:2, :],
                                 start=False, stop=True)
                ot = wpool.tile([128, F], f32, tag="ot")
                nc.scalar.activation(ot[:], ps2[:], ACT, scale=inv2[:])
                nc.sync.dma_start(out[r0:r0 + 128, :], ot[:])

            # Software-pipelined: stage_b of tile i is emitted after stage_a
            # of tile i+1, so the PE fills tile i's LN2-stats gap with tile
            # i+1's transposes + mm1.
            h1_prev = None
            for it in range(n_tiles):
                h1_cur = stage_a(it)
                if h1_prev is not None:
                    stage_b(it - 1, h1_prev)
                h1_prev = h1_cur
            stage_b(n_tiles - 1, h1_prev)

    nc.compile()
    return nc


def _get_nc(rows):
    if rows not in _CACHE:
        _CACHE[rows] = _build(rows)
    return _CACHE[rows]


def _mm_np_dtype():
    if MM_DT == "bfloat16":
        import ml_dtypes
        return ml_dtypes.bfloat16
    return np.float32


def _host_prep(comps, aggr_comps, ln1_g, ln1_b, W1, b1, ln2_g, ln2_b, W2, b2):
    """Fold LN affine params into the weights; resolve gather indices;
    build the [colsum(W); c] rank-1 correction tables."""
    idx = np.searchsorted(np.asarray(aggr_comps), np.asarray(comps)).astype(np.int32)
    W1 = np.asarray(W1, np.float32)
    W2 = np.asarray(W2, np.float32)
    dt = _mm_np_dtype()

    def prep(g, b_ln, W, b_lin):
        wp = (np.asarray(g, np.float32)[:, None] * W).astype(dt)
        c = (np.asarray(b_lin, np.float32)
             + np.asarray(b_ln, np.float32) @ W)
        # colsum of the rounded weights, so the -mu*s term cancels exactly
        # what the device matmul accumulates
        s = wp.astype(np.float32).sum(axis=0)
        sc = np.ascontiguousarray(np.stack([s, c]).astype(dt))
        return np.ascontiguousarray(wp), sc

    w1p, sc1 = prep(ln1_g, ln1_b, W1, b1)
    w2p, sc2 = prep(ln2_g, ln2_b, W2, b2)
    return idx, w1p, sc1, w2p, sc2


def _make_in_maps(nodes, comps, aggr_nodes, aggr_comps,
                  ln1_g, ln1_b, W1, b1, ln2_g, ln2_b, W2, b2):
    nodes = np.asarray(nodes, np.float32)
    aggr_nodes = np.ascontiguousarray(np.asarray(aggr_nodes, np.float32))
    idx, w1p, sc1, w2p, sc2 = _host_prep(
        comps, aggr_comps, ln1_g, ln1_b, W1, b1, ln2_g, ln2_b, W2, b2)
    n = nodes.shape[0]
    if n < N_PAD:
        nodes_p = np.zeros((N_PAD, F), np.float32)
        nodes_p[:n] = nodes
        idx_p = np.zeros((N_PAD,), np.int32)
        idx_p[:n] = idx
    else:
        nodes_p, idx_p = nodes, idx
    n_tiles = ROWS_PER_CORE // 128
    in_maps = []
    for c in range(N_CORES):
        sl = slice(c * ROWS_PER_CORE, (c + 1) * ROWS_PER_CORE)
        in_maps.append({
            "nodes": np.ascontiguousarray(nodes_p[sl]),
            "idx": np.ascontiguousarray(idx_p[sl].reshape(n_tiles, 128).T),
            "aggr": aggr_nodes,
            "w1": w1p, "sc1": sc1, "w2": w2p, "sc2": sc2,
        })
    return in_maps, n


def kernel(coords, nodes, comps, aggr_coords, aggr_nodes, aggr_comps,
           ln1_g, ln1_b, W1, b1, ln2_g, ln2_b, W2, b2):
    from concourse.bass_utils import run_bass_kernel_spmd

    in_maps, n = _make_in_maps(nodes, comps, aggr_nodes, aggr_comps,
                               ln1_g, ln1_b, W1, b1, ln2_g, ln2_b, W2, b2)
    nc = _get_nc(ROWS_PER_CORE)
    res = run_bass_kernel_spmd(nc, in_maps, list(range(N_CORES)))
    out = np.concatenate([res.results[c]["out"] for c in range(N_CORES)], axis=0)
    return out[:n]
